# revision 1
# baseline (speedup 1.0000x reference)
"""GAT forward on 8 Trainium2 NeuronCores (Bass/Tile, SPMD, no collectives).

Sharding: edges assigned to cores by src-node range (N/8 nodes per core);
each core computes the full output rows for its own src range. The small
params (W, a, b) are folded host-side into an augmented weight matrix; the
replicated phase 1 computes per-node rows [h | 1 | v' | u] into a DRAM
table; phase 2 gathers dst rows per edge tile via indirect DMA, builds the
src one-hot on DVE, and scatter-adds [alpha-weighted features | denoms]
into PSUM via the tensor engine. Softmax max-subtraction is skipped:
logits are bounded here so exp is well-conditioned and the result is
mathematically identical.
"""
import math

import numpy as np

N, E, D, P = 50000, 1600000, 128, 128
NCORES = 8
NPC = N // NCORES          # nodes per core
NBLK = math.ceil(NPC / P)  # src blocks per core
ROW = 132                  # row: [h(0:128), ones(128), v'(129), u(130), pad]
PAD_SRC = 999.0
IC = 256                   # index-chunk: tiles per dstT/srcT load

_cache = {}


def _build_program(T, C_logit):
    """Build the SPMD bass program. T = per-block tile counts (len NBLK).
    C_logit = b@(a_src+a_dst) + a_bias, the constant part of every logit
    (the bias b factors out of the alpha-weighted sum since sum(alpha)=1)."""
    from contextlib import ExitStack
    import concourse.bass as bass
    import concourse.bacc as bacc
    import concourse.mybir as mybir
    import concourse.tile as tile
    from concourse.masks import make_identity

    T_total = int(sum(T))
    n_nt = math.ceil(N / P)            # phase-1 node tiles
    nc = bacc.Bacc("TRN2", target_bir_lowering=False, debug=False)

    xT = nc.dram_tensor("xT", [P, n_nt * P], mybir.dt.float32, kind="ExternalInput")
    Wp = nc.dram_tensor("Wp", [P, ROW], mybir.dt.float32, kind="ExternalInput")
    brow = nc.dram_tensor("brow", [1, D], mybir.dt.float32, kind="ExternalInput")
    dstT = nc.dram_tensor("dstT", [P, T_total], mybir.dt.int32, kind="ExternalInput")
    srcT = nc.dram_tensor("srcT", [P, T_total], mybir.dt.float32, kind="ExternalInput")
    uidx = nc.dram_tensor("uidx", [P, NBLK], mybir.dt.int32, kind="ExternalInput")
    out = nc.dram_tensor("out", [NPC, D], mybir.dt.float32, kind="ExternalOutput")

    with tile.TileContext(nc) as tc, ExitStack() as ctx:
        const_p = ctx.enter_context(tc.tile_pool(name="const", bufs=1))
        dram_p = ctx.enter_context(tc.tile_pool(name="dram", bufs=1, space="DRAM"))
        x_p = ctx.enter_context(tc.tile_pool(name="x", bufs=2))
        h_p = ctx.enter_context(tc.tile_pool(name="h", bufs=4))
        ps1_p = ctx.enter_context(tc.tile_pool(name="ps1", bufs=4, space="PSUM"))
        g_p = ctx.enter_context(tc.tile_pool(name="g", bufs=12))
        eq_p = ctx.enter_context(tc.tile_pool(name="eq", bufs=6))
        mex_p = ctx.enter_context(tc.tile_pool(name="mex", bufs=6))
        sm_p = ctx.enter_context(tc.tile_pool(name="sm", bufs=8))
        idx_p = ctx.enter_context(tc.tile_pool(name="idx", bufs=2))
        acc_ps = ctx.enter_context(tc.tile_pool(name="accps", bufs=2, space="PSUM"))
        tr_ps = ctx.enter_context(tc.tile_pool(name="trps", bufs=1, space="PSUM"))
        fin_p = ctx.enter_context(tc.tile_pool(name="fin", bufs=4))

        h_ext = dram_p.tile([N, ROW], mybir.dt.float32)
        assert h_ext[:].offset == 0, "indirect DMA needs zero-offset source"

        # constants
        iota_i = const_p.tile([P, P], mybir.dt.int32)
        nc.gpsimd.iota(iota_i[:], [[1, P]], channel_multiplier=0)
        iota_f = const_p.tile([P, P], mybir.dt.float32)
        nc.vector.tensor_copy(iota_f[:], iota_i[:])
        ident = const_p.tile([P, P], mybir.dt.float32)
        make_identity(nc, ident[:])
        ones_row = const_p.tile([1, P], mybir.dt.float32)
        nc.vector.memset(ones_row[:], 1.0)

        Wp_t = const_p.tile([P, ROW], mybir.dt.float32)
        nc.sync.dma_start(Wp_t[:], Wp[:, :])
        uidx_t = const_p.tile([P, NBLK], mybir.dt.int32)
        nc.sync.dma_start(uidx_t[:], uidx[:, :])
        ones_col = const_p.tile([P, 1], mybir.dt.float32)
        nc.vector.memset(ones_col[:], 1.0)
        c_col = const_p.tile([P, 1], mybir.dt.float32)
        nc.vector.memset(c_col[:], float(C_logit))
        c001_col = const_p.tile([P, 1], mybir.dt.float32)
        nc.vector.memset(c001_col[:], float(0.01 * C_logit))
        # b_mat[p, f] = b[f] (bias re-added after aggregation: sum(alpha)=1)
        b_row = const_p.tile([1, D], mybir.dt.float32)
        nc.sync.dma_start(b_row[:], brow[:, :])
        b_psum = tr_ps.tile([P, P], mybir.dt.float32, tag="umatp")
        nc.tensor.matmul(b_psum[:, :D], lhsT=ones_row[:], rhs=b_row[:],
                         start=True, stop=True)
        b_mat = const_p.tile([P, D], mybir.dt.float32)
        nc.vector.tensor_copy(b_mat[:], b_psum[:, :D])

        # ---- phase 1: h' = x @ W' + b' -> h_ext ----
        XC = 16  # node tiles per x chunk
        for c0 in range(0, n_nt, XC):
            cn = min(XC, n_nt - c0)
            xc = x_p.tile([P, XC * P], mybir.dt.float32, tag="xc")
            nc.sync.dma_start(xc[:, :cn * P], xT[:, c0 * P:(c0 + cn) * P])
            for j in range(cn):
                ps = ps1_p.tile([P, ROW], mybir.dt.float32, tag="ps1")
                nc.tensor.matmul(ps[:], lhsT=xc[:, j * P:(j + 1) * P],
                                 rhs=Wp_t[:], start=True, stop=True)
                ht = h_p.tile([P, ROW], mybir.dt.float32, tag="ht")
                nc.scalar.copy(ht[:], ps[:])
                nc.vector.memset(ht[:, 128:129], 1.0)
                nt = c0 + j
                rows = min(P, N - nt * P)
                nc.sync.dma_start(h_ext[nt * P:nt * P + rows, :], ht[:rows, :])

        h_flat = h_ext[:].rearrange("n (r o) -> (n r) o", o=1)

        # ---- phase 2: edge tiles ----
        # flatten (block, tile) schedule with chunked index loads
        sched = []  # (block, tt, t)
        t = 0
        for b in range(NBLK):
            for tt in range(T[b]):
                sched.append((b, tt, t))
                t += 1
        chunk_tiles = {}

        def get_chunk(t):
            c0 = (t // IC) * IC
            if c0 not in chunk_tiles:
                cn = min(IC, T_total - c0)
                dst_c = idx_p.tile([P, IC], mybir.dt.int32, tag="dstc")
                nc.sync.dma_start(dst_c[:, :cn], dstT[:, c0:c0 + cn])
                src_c = idx_p.tile([P, IC], mybir.dt.float32, tag="srcc")
                nc.sync.dma_start(src_c[:, :cn], srcT[:, c0:c0 + cn])
                chunk_tiles[c0] = (dst_c, src_c)
            return chunk_tiles[c0], t - c0

        cur_block = -1
        acc = None
        u_mat = None
        for (b, tt, t) in sched:
            if b != cur_block:
                cur_block = b
                u_blk = sm_p.tile([P, 1], mybir.dt.float32, tag="ublk")
                nc.gpsimd.indirect_dma_start(
                    out=u_blk[:], out_offset=None, in_=h_flat,
                    in_offset=bass.IndirectOffsetOnAxis(
                        ap=uidx_t[:, b:b + 1], axis=0))
                # u_mat[p, i] = u_blk[i]  (broadcast along partitions)
                u_rowp = tr_ps.tile([P, P], mybir.dt.float32, tag="urowp")
                nc.tensor.transpose(u_rowp[:1, :], u_blk[:], ident[:])
                u_row = sm_p.tile([1, P], mybir.dt.float32, tag="urow")
                nc.vector.tensor_copy(u_row[:], u_rowp[:1, :])
                u_matp = tr_ps.tile([P, P], mybir.dt.float32, tag="umatp")
                nc.tensor.matmul(u_matp[:], lhsT=ones_row[:], rhs=u_row[:],
                                 start=True, stop=True)
                u_mat = mex_p.tile([P, P], mybir.dt.float32, tag="umat")
                nc.vector.tensor_copy(u_mat[:], u_matp[:])
                acc = acc_ps.tile([P, D + 1], mybir.dt.float32, tag="acc")
            (dst_c, src_c), t_loc = get_chunk(t)
            g_t = g_p.tile([P, ROW], mybir.dt.float32, tag="g")
            nc.gpsimd.indirect_dma_start(
                out=g_t[:], out_offset=None, in_=h_ext[:, :],
                in_offset=bass.IndirectOffsetOnAxis(
                    ap=dst_c[:, t_loc:t_loc + 1], axis=0))
            # S[e,i] = u[i] + v'[e]; exp(leaky_relu(s)) == max(exp(s), exp(.01 s))
            eq = eq_p.tile([P, P], mybir.dt.float32, tag="eq")
            nc.vector.tensor_scalar(
                out=eq[:], in0=iota_f[:], scalar1=src_c[:, t_loc:t_loc + 1],
                scalar2=None, op0=mybir.AluOpType.is_equal)
            s_m = eq_p.tile([P, P], mybir.dt.float32, tag="sm")
            nc.vector.tensor_scalar(
                out=s_m[:], in0=u_mat[:], scalar1=g_t[:, 129:130],
                scalar2=None, op0=mybir.AluOpType.add)
            ex1 = mex_p.tile([P, P], mybir.dt.float32, tag="ex1")
            nc.scalar.activation(ex1[:], s_m[:],
                                 mybir.ActivationFunctionType.Exp,
                                 bias=c_col[:])
            ex2 = mex_p.tile([P, P], mybir.dt.float32, tag="ex2")
            nc.scalar.activation(ex2[:], s_m[:],
                                 mybir.ActivationFunctionType.Exp,
                                 bias=c001_col[:], scale=0.01)
            raw = eq_p.tile([P, P], mybir.dt.float32, tag="raw")
            nc.vector.tensor_tensor(out=raw[:], in0=ex1[:], in1=ex2[:],
                                    op=mybir.AluOpType.max)
            mex = mex_p.tile([P, P], mybir.dt.float32, tag="mex")
            nc.vector.tensor_tensor(out=mex[:], in0=raw[:], in1=eq[:],
                                    op=mybir.AluOpType.mult)
            nc.tensor.matmul(acc[:], lhsT=mex[:], rhs=g_t[:, 0:D + 1],
                             start=(tt == 0), stop=(tt == T[b] - 1))
            if tt == T[b] - 1:
                rows_b = min(P, NPC - b * P)
                recip = sm_p.tile([P, 1], mybir.dt.float32, tag="recip")
                nc.vector.reciprocal(recip[:], acc[:, D:D + 1])
                scaled = fin_p.tile([P, D], mybir.dt.float32, tag="scaled")
                nc.vector.tensor_scalar(out=scaled[:], in0=acc[:, 0:D],
                                        scalar1=recip[:], scalar2=None,
                                        op0=mybir.AluOpType.mult)
                sb = fin_p.tile([P, D], mybir.dt.float32, tag="sb")
                nc.vector.tensor_tensor(out=sb[:], in0=scaled[:], in1=b_mat[:],
                                        op=mybir.AluOpType.add)
                scaled = sb
                mn = fin_p.tile([P, D], mybir.dt.float32, tag="mn")
                nc.vector.tensor_scalar(out=mn[:], in0=scaled[:], scalar1=0.0,
                                        scalar2=None, op0=mybir.AluOpType.min)
                em = fin_p.tile([P, D], mybir.dt.float32, tag="em")
                nc.scalar.activation(em[:], mn[:],
                                     mybir.ActivationFunctionType.Exp)
                t1 = fin_p.tile([P, D], mybir.dt.float32, tag="t1")
                nc.vector.tensor_tensor(out=t1[:], in0=scaled[:], in1=mn[:],
                                        op=mybir.AluOpType.subtract)
                t2 = fin_p.tile([P, D], mybir.dt.float32, tag="t2")
                nc.vector.tensor_tensor(out=t2[:], in0=t1[:], in1=em[:],
                                        op=mybir.AluOpType.add)
                ot = fin_p.tile([P, D], mybir.dt.float32, tag="ot")
                nc.vector.tensor_scalar(out=ot[:], in0=t2[:], scalar1=-1.0,
                                        scalar2=None, op0=mybir.AluOpType.add)
                nc.sync.dma_start(out[b * P:b * P + rows_b, :], ot[:rows_b, :])

    nc.compile()
    return nc


def _prep(x, edge_index, W, b, a, a_bias):
    """Host-side sharding/layout. Returns (T, per-core input maps)."""
    x = np.asarray(x, np.float32)
    ei = np.asarray(edge_index)
    W = np.asarray(W, np.float32)
    b = np.asarray(b, np.float32)
    a = np.asarray(a, np.float32)
    a_bias = float(np.asarray(a_bias))

    a_src, a_dst = a[:D], a[D:]
    Wp = np.zeros((P, ROW), np.float32)
    Wp[:, :D] = W
    Wp[:, 129] = W @ a_dst
    Wp[:, 130] = W @ a_src
    C_logit = float(b @ a_dst + b @ a_src + a_bias)
    brow = b.reshape(1, D).astype(np.float32)

    sl = np.arange(N, dtype=np.int64)
    src = np.concatenate([ei[0].astype(np.int64), sl])
    dst = np.concatenate([ei[1].astype(np.int64), sl])
    order = np.argsort(src, kind="stable")
    src = src[order]
    dst = dst[order]
    counts = np.zeros((NCORES, NBLK), np.int64)
    core_all = src // NPC
    blk_all = (src - core_all * NPC) // P
    np.add.at(counts, (core_all, blk_all), 1)
    T = np.maximum((counts + P - 1) // P, 1).max(axis=0)
    T_total = int(T.sum())

    cstart = np.searchsorted(src, np.arange(NCORES) * NPC)
    cend = np.append(cstart[1:], len(src))
    n_nt = math.ceil(N / P)
    xT = np.zeros((P, n_nt * P), np.float32)
    xT[:, :N] = x.T

    core_inputs = []
    for c in range(NCORES):
        s, e = cstart[c], cend[c]
        csrc, cdst = src[s:e], dst[s:e]
        lblk = (csrc - c * NPC) // P
        bstart = np.searchsorted(lblk, np.arange(NBLK))
        bend = np.append(bstart[1:], len(csrc))
        dst_arr = np.zeros((T_total, P), np.int32)
        src_arr = np.full((T_total, P), PAD_SRC, np.float32)
        t0 = 0
        for bb in range(NBLK):
            bs, be = int(bstart[bb]), int(bend[bb])
            nbe = be - bs
            d_pad = np.zeros(int(T[bb]) * P, np.int32)
            s_pad = np.full(int(T[bb]) * P, PAD_SRC, np.float32)
            d_pad[:nbe] = cdst[bs:be]
            s_pad[:nbe] = (csrc[bs:be] - c * NPC - bb * P).astype(np.float32)
            dst_arr[t0:t0 + int(T[bb])] = d_pad.reshape(int(T[bb]), P)
            src_arr[t0:t0 + int(T[bb])] = s_pad.reshape(int(T[bb]), P)
            t0 += int(T[bb])
        u_idx = np.zeros((NBLK, P), np.int32)
        for bb in range(NBLK):
            rows = np.minimum(c * NPC + bb * P + np.arange(P), N - 1)
            u_idx[bb] = (rows * ROW + 130).astype(np.int32)
        core_inputs.append({
            "xT": xT,
            "Wp": Wp,
            "brow": brow,
            "dstT": np.ascontiguousarray(dst_arr.T),
            "srcT": np.ascontiguousarray(src_arr.T),
            "uidx": np.ascontiguousarray(u_idx.T.astype(np.int32)),
        })
    return T, core_inputs, C_logit


_last_results = None


def kernel(x, edge_index, W, b, a, a_bias):
    global _last_results
    from concourse.bass_utils import run_bass_kernel_spmd

    T, core_inputs, C_logit = _prep(x, edge_index, W, b, a, a_bias)
    key = (tuple(int(v) for v in T), round(C_logit, 12))
    if key not in _cache:
        _cache[key] = _build_program(T, C_logit)
    nc = _cache[key]
    res = run_bass_kernel_spmd(nc, core_inputs, core_ids=list(range(NCORES)),
                               trace=False)
    _last_results = res
    outs = [res.results[c]["out"] for c in range(NCORES)]
    return np.concatenate(outs, axis=0)

